# revision 6
# baseline (speedup 1.0000x reference)
"""ColorLoss Trainium2 kernel v3: interleaved-pair encodings.

Both e1 (stationary) and e2 (moving) use layout [p, th, 64, 2] — innermost
dim is the 2-pixel-block pair, so:
  - matmul operand slices [p, tau, :, :] merge to a single contiguous
    [p, 128] free dim -> fast LDWEIGHTS (FWL) + fast moving stream.
  - broadcast is_equal generation on DVE has unit innermost strides on all
    three APs (key pairs, doubled iota, dst) -> 2x DVE mode.
PSUM indices: M = 2*j1 + b, N = 2*j2 + b'; host sums the b-diagonal.
ACT engine: preprocessing Relu chains + a few e1 Sign planes (strided dst).
"""
import sys

sys.path.insert(0, "/opt/trn_rl_repo")
import numpy as np
from contextlib import ExitStack

import ml_dtypes  # noqa: F401

# ---------------- tunables ----------------
T = 320            # pixels per partition per chunk (must be even)
N_ACT_PLANES = 30  # e1 planes on ScalarE (Sign, strided dst)
H, W = 1024, 1024
HW = H * W
IMG_PP = HW // 128
STY_PP = 128 * W // 128

_cache = {}


def _plane_sets():
    assert N_ACT_PLANES % 2 == 0, "sign-plane count must be even"
    act = set(range(2, 2 + N_ACT_PLANES))
    k = np.arange(64)
    M1 = np.eye(64)
    for j in act:
        M1[j] = np.sign(k - j)
    cond = np.linalg.cond(M1)
    assert cond < 1e8, f"bad plane split, cond={cond}"
    return act, np.linalg.inv(M1)


def _build():
    import concourse.bacc as bacc
    import concourse.mybir as mybir
    from concourse.tile import TileContext

    F32 = mybir.dt.float32
    BF16 = mybir.dt.bfloat16
    I16 = mybir.dt.int16
    Alu = mybir.AluOpType
    Act = mybir.ActivationFunctionType

    act_set, M1inv = _plane_sets()
    dve_js = [j for j in range(64) if j not in act_set]
    # dve planes must form contiguous ranges for few TT calls: group runs
    runs = []
    for j in dve_js:
        if runs and runs[-1][1] == j:
            runs[-1][1] = j + 1
        else:
            runs.append([j, j + 1])

    nc = bacc.Bacc("TRN2")
    img_d = nc.dram_tensor("img", [3, H, W], F32, kind="ExternalInput")
    sty_d = nc.dram_tensor("sty", [3, 128, W], F32, kind="ExternalInput")
    o_d = nc.dram_tensor("out", [2, 128, 128], F32, kind="ExternalOutput")

    img_v = [img_d[c, :, :].rearrange("(p r) w -> p (r w)", p=128)
             for c in range(3)]
    sty_v = [sty_d[c, :, :] for c in range(3)]

    def chunks(total):
        off, out = 0, []
        while off < total:
            tc_ = min(T, total - off)
            out.append((off, tc_))
            off += tc_
        return out

    img_chunks = chunks(IMG_PP)
    sty_chunks = chunks(STY_PP)

    with TileContext(nc) as tc:
        with ExitStack() as ctx:
            xpool = ctx.enter_context(tc.tile_pool(name="x", bufs=3))
            tpool = ctx.enter_context(tc.tile_pool(name="t", bufs=2))
            ipool = ctx.enter_context(tc.tile_pool(name="i", bufs=2))
            kpool = ctx.enter_context(tc.tile_pool(name="k", bufs=2))
            e1pool = ctx.enter_context(tc.tile_pool(name="e1", bufs=2))
            e2pool = ctx.enter_context(tc.tile_pool(name="e2", bufs=2))
            cpool = ctx.enter_context(tc.tile_pool(name="c", bufs=1))
            opool = ctx.enter_context(tc.tile_pool(name="o", bufs=1))
            pspool = ctx.enter_context(tc.tile_pool(name="ps", bufs=2,
                                                    space="PSUM"))

            # constants
            iota2 = cpool.tile([128, 64, 2], BF16, tag="iota2")
            for j in range(64):
                nc.vector.memset(iota2[:, j, :], float(j))
            bcl1 = cpool.tile([128, 1], F32, tag="bcl1")
            nc.vector.memset(bcl1[:], 7.4)
            bcl3 = cpool.tile([128, 1], F32, tag="bcl3")
            nc.vector.memset(bcl3[:], 14.9)
            bcl4 = cpool.tile([128, 1], F32, tag="bcl4")
            nc.vector.memset(bcl4[:], 3.35)
            bias_j = cpool.tile([128, 64], F32, tag="biasj")
            for j in act_set:
                nc.vector.memset(bias_j[:, j:j + 1], -float(j))

            ps_img = pspool.tile([128, 128], F32)
            ps_sty = pspool.tile([128, 128], F32)

            def do_chunk(views, off, tcw, ps, start, stop):
                th = tcw // 2
                xt = xpool.tile([128, 3, T], F32, tag="xt")
                for c in range(3):
                    nc.sync.dma_start(xt[:, c, :tcw], views[c][:, off:off + tcw])
                ut = tpool.tile([128, 3, T], F32, tag="ut")
                for c in range(3):
                    nc.scalar.activation(ut[:, c, :tcw], xt[:, c, :tcw],
                                         Act.Relu, bias=bcl1[:], scale=-8.0)
                ii = ipool.tile([128, 4, T], I16, tag="ii")
                # ii_c = round(Relu(-ut+15.4)-0.5) == Relu(-ut+14.9) -> i16 RNE
                for c in range(3):
                    nc.scalar.activation(ii[:, c, :tcw], ut[:, c, :tcw],
                                         Act.Relu, bias=bcl3[:], scale=-1.0)
                # ii3 = floor(g/4) == Relu(-0.25*ut_g + 3.35) -> i16 RNE
                nc.scalar.activation(ii[:, 3, :tcw], ut[:, 1, :tcw],
                                     Act.Relu, bias=bcl4[:], scale=-0.25)
                # gl = g - 4*gh straight from int16 ii (DVE converts on read)
                gl = kpool.tile([128, T], BF16, tag="gl")
                nc.vector.scalar_tensor_tensor(gl[:, :tcw], ii[:, 3, :tcw],
                                               -4.0, ii[:, 1, :tcw],
                                               Alu.mult, Alu.add)
                # pair-shuffled keys written directly: kp[p,c,t,b] = key_c[p, b*th+t]
                kp = kpool.tile([128, 2, T // 2, 2], BF16, tag="kp")
                kp1 = kp[:, 0, :th, :].rearrange("p t b -> p b t")
                kp2 = kp[:, 1, :th, :].rearrange("p t b -> p b t")
                glv = gl[:, :tcw].rearrange("p (b t) -> p b t", b=2)
                rv = ii[:, 0, :tcw].rearrange("p (b t) -> p b t", b=2)
                bv = ii[:, 2, :tcw].rearrange("p (b t) -> p b t", b=2)
                ghv = ii[:, 3, :tcw].rearrange("p (b t) -> p b t", b=2)
                nc.vector.scalar_tensor_tensor(kp1, glv, 16.0, rv,
                                               Alu.mult, Alu.add)
                nc.vector.scalar_tensor_tensor(kp2, bv, 4.0, ghv,
                                               Alu.mult, Alu.add)

                # interleaved encodings [p, th, 64, 2]
                e1 = e1pool.tile([128, T // 2, 64, 2], BF16, tag="e1")
                e2 = e2pool.tile([128, T // 2, 64, 2], BF16, tag="e2")
                k1b = kp[:, 0, :th, :].unsqueeze(2)
                k2b = kp[:, 1, :th, :].unsqueeze(2)
                i2b = iota2.unsqueeze(1)
                # e2: full 64 planes in one 2x TT
                nc.vector.tensor_tensor(
                    e2[:, :th, :, :],
                    k2b.broadcast_to([128, th, 64, 2]),
                    i2b.broadcast_to([128, th, 64, 2]), Alu.is_equal)
                # e1: DVE runs + ACT planes
                for j0, j1 in runs:
                    w = j1 - j0
                    nc.vector.tensor_tensor(
                        e1[:, :th, j0:j1, :],
                        k1b.broadcast_to([128, th, w, 2]),
                        i2b[:, :, j0:j1, :].broadcast_to([128, th, w, 2]),
                        Alu.is_equal)
                for j in act_set:
                    nc.scalar.activation(e1[:, :th, j, :], kp[:, 0, :th, :],
                                         Act.Sign, bias=bias_j[:, j:j + 1],
                                         scale=1.0)

                for tau in range(th):
                    nc.tensor.matmul(
                        ps[:],
                        e1[:, tau, :, :],
                        e2[:, tau, :, :],
                        start=(start and tau == 0),
                        stop=(stop and tau == th - 1),
                    )

            n_img = len(img_chunks)
            for ci, (off, tcw) in enumerate(img_chunks):
                do_chunk(img_v, off, tcw, ps_img, ci == 0, ci == n_img - 1)
            n_sty = len(sty_chunks)
            for ci, (off, tcw) in enumerate(sty_chunks):
                do_chunk(sty_v, off, tcw, ps_sty, ci == 0, ci == n_sty - 1)

            ostage = opool.tile([128, 2, 128], F32)
            nc.vector.tensor_copy(ostage[:, 0, :], ps_img[:])
            nc.vector.tensor_copy(ostage[:, 1, :], ps_sty[:])
            nc.sync.dma_start(o_d[0, :, :], ostage[:, 0, :])
            nc.sync.dma_start(o_d[1, :, :], ostage[:, 1, :])

    nc.finalize()
    return nc, M1inv


def _get_built():
    if "nc" not in _cache:
        nc, M1inv = _build()
        _cache["nc"] = nc
        _cache["M1inv"] = M1inv
    return _cache["nc"], _cache["M1inv"]


def _unmix(raw, M1inv):
    """raw [2,128,128] f32, M=2*j1+b, N=2*j2+b' -> exact count matrices."""
    out = []
    for s in range(2):
        r = raw[s].astype(np.float64)
        mixed = r[0::2, 0::2] + r[1::2, 1::2]   # [64 j1, 64 j2]
        Hm = M1inv @ mixed
        out.append(np.rint(Hm))
    return out


def kernel(input, style_image, n_bins):
    assert int(n_bins) == 16
    from concourse import bass_utils

    nc, M1inv = _get_built()
    input = np.ascontiguousarray(np.asarray(input, dtype=np.float32))
    style = np.ascontiguousarray(np.asarray(style_image, dtype=np.float32))
    B = input.shape[0]
    assert B == 8 and input.shape == (8, 3, H, W)
    in_maps = [
        {
            "img": input[i],
            "sty": np.ascontiguousarray(style[0, :, 128 * i:128 * (i + 1), :]),
        }
        for i in range(8)
    ]
    res = bass_utils.run_bass_kernel_spmd(nc, in_maps, core_ids=list(range(8)),
                                          **_cache.get("run_kwargs", {}))
    _cache["last_results"] = res
    hists = np.zeros((B, 4096), np.float64)
    sty_hist = np.zeros(4096, np.float64)
    for i in range(8):
        hi, hs = _unmix(res.results[i]["out"], M1inv)
        # flat = key1 + 64*key2 -> hist_flat[f] = H[j1=f%64, j2=f//64]
        hists[i] = hi.T.reshape(4096)
        sty_hist += hs.T.reshape(4096)
    cols = (hists / HW).astype(np.float32)
    target = (sty_hist / HW).astype(np.float32)
    loss = np.mean(np.abs(cols - target[None, :]).astype(np.float32))
    return np.float32(loss)


# revision 7
# speedup vs baseline: 1.0004x; 1.0004x over previous
"""ColorLoss Trainium2 kernel v3: interleaved-pair encodings.

Both e1 (stationary) and e2 (moving) use layout [p, th, 64, 2] — innermost
dim is the 2-pixel-block pair, so:
  - matmul operand slices [p, tau, :, :] merge to a single contiguous
    [p, 128] free dim -> fast LDWEIGHTS (FWL) + fast moving stream.
  - broadcast is_equal generation on DVE has unit innermost strides on all
    three APs (key pairs, doubled iota, dst) -> 2x DVE mode.
PSUM indices: M = 2*j1 + b, N = 2*j2 + b'; host sums the b-diagonal.
ACT engine: preprocessing Relu chains + a few e1 Sign planes (strided dst).
"""
import sys

sys.path.insert(0, "/opt/trn_rl_repo")
import numpy as np
from contextlib import ExitStack

import ml_dtypes  # noqa: F401

# ---------------- tunables ----------------
T = 288            # pixels per partition per chunk (must be even)
N_ACT_PLANES = 30  # e1 planes on ScalarE (Sign, strided dst)
H, W = 1024, 1024
HW = H * W
IMG_PP = HW // 128
STY_PP = 128 * W // 128

_cache = {}


def _plane_sets():
    assert N_ACT_PLANES % 2 == 0, "sign-plane count must be even"
    act = set(range(2, 2 + N_ACT_PLANES))
    k = np.arange(64)
    M1 = np.eye(64)
    for j in act:
        M1[j] = np.sign(k - j)
    cond = np.linalg.cond(M1)
    assert cond < 1e8, f"bad plane split, cond={cond}"
    return act, np.linalg.inv(M1)


def _build():
    import concourse.bacc as bacc
    import concourse.mybir as mybir
    from concourse.tile import TileContext

    F32 = mybir.dt.float32
    BF16 = mybir.dt.bfloat16
    I16 = mybir.dt.int16
    Alu = mybir.AluOpType
    Act = mybir.ActivationFunctionType

    act_set, M1inv = _plane_sets()
    dve_js = [j for j in range(64) if j not in act_set]
    # dve planes must form contiguous ranges for few TT calls: group runs
    runs = []
    for j in dve_js:
        if runs and runs[-1][1] == j:
            runs[-1][1] = j + 1
        else:
            runs.append([j, j + 1])

    nc = bacc.Bacc("TRN2")
    img_d = nc.dram_tensor("img", [3, H, W], F32, kind="ExternalInput")
    sty_d = nc.dram_tensor("sty", [3, 128, W], F32, kind="ExternalInput")
    o_d = nc.dram_tensor("out", [2, 128, 128], F32, kind="ExternalOutput")

    img_v = [img_d[c, :, :].rearrange("(p r) w -> p (r w)", p=128)
             for c in range(3)]
    sty_v = [sty_d[c, :, :] for c in range(3)]

    def chunks(total):
        off, out = 0, []
        while off < total:
            tc_ = min(T, total - off)
            out.append((off, tc_))
            off += tc_
        return out

    img_chunks = chunks(IMG_PP)
    sty_chunks = chunks(STY_PP)

    with TileContext(nc) as tc:
        with ExitStack() as ctx:
            xpool = ctx.enter_context(tc.tile_pool(name="x", bufs=3))
            tpool = ctx.enter_context(tc.tile_pool(name="t", bufs=2))
            ipool = ctx.enter_context(tc.tile_pool(name="i", bufs=2))
            kpool = ctx.enter_context(tc.tile_pool(name="k", bufs=2))
            e1pool = ctx.enter_context(tc.tile_pool(name="e1", bufs=2))
            e2pool = ctx.enter_context(tc.tile_pool(name="e2", bufs=2))
            cpool = ctx.enter_context(tc.tile_pool(name="c", bufs=1))
            opool = ctx.enter_context(tc.tile_pool(name="o", bufs=1))
            pspool = ctx.enter_context(tc.tile_pool(name="ps", bufs=2,
                                                    space="PSUM"))

            # constants
            iota2 = cpool.tile([128, 64, 2], BF16, tag="iota2")
            for j in range(64):
                nc.vector.memset(iota2[:, j, :], float(j))
            bcl1 = cpool.tile([128, 1], F32, tag="bcl1")
            nc.vector.memset(bcl1[:], 7.4)
            bcl3 = cpool.tile([128, 1], F32, tag="bcl3")
            nc.vector.memset(bcl3[:], 14.9)
            bcl4 = cpool.tile([128, 1], F32, tag="bcl4")
            nc.vector.memset(bcl4[:], 3.35)
            bias_j = cpool.tile([128, 64], F32, tag="biasj")
            for j in act_set:
                nc.vector.memset(bias_j[:, j:j + 1], -float(j))

            ps_img = pspool.tile([128, 128], F32)
            ps_sty = pspool.tile([128, 128], F32)

            def do_chunk(views, off, tcw, ps, start, stop):
                th = tcw // 2
                xt = xpool.tile([128, 3, T], F32, tag="xt")
                for c in range(3):
                    nc.sync.dma_start(xt[:, c, :tcw], views[c][:, off:off + tcw])
                ut = tpool.tile([128, 3, T], F32, tag="ut")
                for c in range(3):
                    nc.scalar.activation(ut[:, c, :tcw], xt[:, c, :tcw],
                                         Act.Relu, bias=bcl1[:], scale=-8.0)
                ii = ipool.tile([128, 4, T], I16, tag="ii")
                # ii_c = round(Relu(-ut+15.4)-0.5) == Relu(-ut+14.9) -> i16 RNE
                for c in range(3):
                    nc.scalar.activation(ii[:, c, :tcw], ut[:, c, :tcw],
                                         Act.Relu, bias=bcl3[:], scale=-1.0)
                # ii3 = floor(g/4) == Relu(-0.25*ut_g + 3.35) -> i16 RNE
                nc.scalar.activation(ii[:, 3, :tcw], ut[:, 1, :tcw],
                                     Act.Relu, bias=bcl4[:], scale=-0.25)
                # gl = g - 4*gh straight from int16 ii (DVE converts on read)
                gl = kpool.tile([128, T], BF16, tag="gl")
                nc.vector.scalar_tensor_tensor(gl[:, :tcw], ii[:, 3, :tcw],
                                               -4.0, ii[:, 1, :tcw],
                                               Alu.mult, Alu.add)
                # pair-shuffled keys written directly: kp[p,c,t,b] = key_c[p, b*th+t]
                kp = kpool.tile([128, 2, T // 2, 2], BF16, tag="kp")
                kp1 = kp[:, 0, :th, :].rearrange("p t b -> p b t")
                kp2 = kp[:, 1, :th, :].rearrange("p t b -> p b t")
                glv = gl[:, :tcw].rearrange("p (b t) -> p b t", b=2)
                rv = ii[:, 0, :tcw].rearrange("p (b t) -> p b t", b=2)
                bv = ii[:, 2, :tcw].rearrange("p (b t) -> p b t", b=2)
                ghv = ii[:, 3, :tcw].rearrange("p (b t) -> p b t", b=2)
                nc.vector.scalar_tensor_tensor(kp1, glv, 16.0, rv,
                                               Alu.mult, Alu.add)
                nc.vector.scalar_tensor_tensor(kp2, bv, 4.0, ghv,
                                               Alu.mult, Alu.add)

                # interleaved encodings [p, th, 64, 2]
                e1 = e1pool.tile([128, T // 2, 64, 2], BF16, tag="e1")
                e2 = e2pool.tile([128, T // 2, 64, 2], BF16, tag="e2")
                k1b = kp[:, 0, :th, :].unsqueeze(2)
                k2b = kp[:, 1, :th, :].unsqueeze(2)
                i2b = iota2.unsqueeze(1)
                # e2: full 64 planes in one 2x TT
                nc.vector.tensor_tensor(
                    e2[:, :th, :, :],
                    k2b.broadcast_to([128, th, 64, 2]),
                    i2b.broadcast_to([128, th, 64, 2]), Alu.is_equal)
                # e1: DVE runs + ACT planes
                for j0, j1 in runs:
                    w = j1 - j0
                    nc.vector.tensor_tensor(
                        e1[:, :th, j0:j1, :],
                        k1b.broadcast_to([128, th, w, 2]),
                        i2b[:, :, j0:j1, :].broadcast_to([128, th, w, 2]),
                        Alu.is_equal)
                for j in act_set:
                    nc.scalar.activation(e1[:, :th, j, :], kp[:, 0, :th, :],
                                         Act.Sign, bias=bias_j[:, j:j + 1],
                                         scale=1.0)

                for tau in range(th):
                    nc.tensor.matmul(
                        ps[:],
                        e1[:, tau, :, :],
                        e2[:, tau, :, :],
                        start=(start and tau == 0),
                        stop=(stop and tau == th - 1),
                    )

            n_img = len(img_chunks)
            for ci, (off, tcw) in enumerate(img_chunks):
                do_chunk(img_v, off, tcw, ps_img, ci == 0, ci == n_img - 1)
            n_sty = len(sty_chunks)
            for ci, (off, tcw) in enumerate(sty_chunks):
                do_chunk(sty_v, off, tcw, ps_sty, ci == 0, ci == n_sty - 1)

            ostage = opool.tile([128, 2, 128], F32)
            nc.vector.tensor_copy(ostage[:, 0, :], ps_img[:])
            nc.vector.tensor_copy(ostage[:, 1, :], ps_sty[:])
            nc.sync.dma_start(o_d[0, :, :], ostage[:, 0, :])
            nc.sync.dma_start(o_d[1, :, :], ostage[:, 1, :])

    nc.finalize()
    return nc, M1inv


def _get_built():
    if "nc" not in _cache:
        nc, M1inv = _build()
        _cache["nc"] = nc
        _cache["M1inv"] = M1inv
    return _cache["nc"], _cache["M1inv"]


def _unmix(raw, M1inv):
    """raw [2,128,128] f32, M=2*j1+b, N=2*j2+b' -> exact count matrices."""
    out = []
    for s in range(2):
        r = raw[s].astype(np.float64)
        mixed = r[0::2, 0::2] + r[1::2, 1::2]   # [64 j1, 64 j2]
        Hm = M1inv @ mixed
        out.append(np.rint(Hm))
    return out


def kernel(input, style_image, n_bins):
    assert int(n_bins) == 16
    from concourse import bass_utils

    nc, M1inv = _get_built()
    input = np.ascontiguousarray(np.asarray(input, dtype=np.float32))
    style = np.ascontiguousarray(np.asarray(style_image, dtype=np.float32))
    B = input.shape[0]
    assert B == 8 and input.shape == (8, 3, H, W)
    in_maps = [
        {
            "img": input[i],
            "sty": np.ascontiguousarray(style[0, :, 128 * i:128 * (i + 1), :]),
        }
        for i in range(8)
    ]
    res = bass_utils.run_bass_kernel_spmd(nc, in_maps, core_ids=list(range(8)),
                                          **_cache.get("run_kwargs", {}))
    _cache["last_results"] = res
    hists = np.zeros((B, 4096), np.float64)
    sty_hist = np.zeros(4096, np.float64)
    for i in range(8):
        hi, hs = _unmix(res.results[i]["out"], M1inv)
        # flat = key1 + 64*key2 -> hist_flat[f] = H[j1=f%64, j2=f//64]
        hists[i] = hi.T.reshape(4096)
        sty_hist += hs.T.reshape(4096)
    cols = (hists / HW).astype(np.float32)
    target = (sty_hist / HW).astype(np.float32)
    loss = np.mean(np.abs(cols - target[None, :]).astype(np.float32))
    return np.float32(loss)
